# revision 11
# baseline (speedup 1.0000x reference)
"""AdaptiveEmbedding kernel for 8 TRN2 NeuronCores (host-gather GEMM,
int8 output). v6: all-bf16, staged input chunks, deferred weight DMAs,
half-tile casts on vector+scalar, 8 PSUM banks, 8 output groups.

Host routes tokens to vocab buckets and gathers their embedding rows into
dense feature-on-partition tiles (token-parallel across 8 cores, projection
weights replicated). Device is a pure pipelined GEMM: per 128-token tile,
stationary = gathered embeddings [K_feat, 128tok], moving = projection
[K_feat, 512 dproj], PSUM [tok, 512] f32 per half, then a scaled
round+saturate cast to int8, DMA out in p-major layout (row = p*16+slot,
2KB contiguous per partition per 2-tile group). Host dequantizes and
scatters rows back to token order.

Norm note: bucket 0 tokens carry ~54% of the output norm (d_emb=1024),
so fp8 there (3.7% rel) costs ~2.4e-2 global -- everything stays bf16.

Scheduling: the z-critical transfers (ez first chunk, wz halves) issue
first on the sync HWDGE ring; e1/w1 go on the scalar ring; the big w0
halves and e0 are issued mid-loop (after groups 0/1/2) so their transfers
never contend with the z stream. PE warmup matmuls bridge the queue from
preamble end (~7.7us) to first data (~9.4us) to keep the HAM activity
window warm. gpsimd issues no DMAs (SWDGE quiesces against all in-flight
DMA) and cannot read PSUM, so casts alternate vector/scalar per 512-col
half-tile with 8 single-bank PSUM tiles in flight.

Routing overflow beyond the static caps falls back to exact numpy on host.
Self-contained: shapes hardcoded.
"""

import numpy as np
import ml_dtypes

BF16 = ml_dtypes.bfloat16

CUT = [0, 20000, 40000, 200000, 267735]
D_EMBS = [1024, 256, 64, 16]
D_PROJ = 1024
NCORES = 8
P = 128

CAP0 = 128     # b0: mean 153 — overflow (~25/core) goes to exact host fallback
CAP1 = 128     # b1: same
CAPZ = 1792    # b2+b3 merged: mean 1742, sigma ~16; 14 full 128-token tiles
NSIG = 5.0     # quantization range in output sigmas

# tile ids: 0..13 = z tiles, 14 = b1, 15 = b0. b1 sits mid-stream; b0 runs
# LAST so its (late-arriving, largest) weight stream never stalls the
# in-order tensor queue.
ORDER = [0, 1, 2, 3,  4, 5, 14, 6,  7, 8, 9, 10,  11, 12, 13, 15]
NGROUPS = 8                                  # 8 output groups x 2 tiles
SLOT = {t: s for s, t in enumerate(ORDER)}   # tile id -> slot
OUT_ROWS = 16 * P                            # 2048

# cast engine per 512-col half-tile: vector takes h=0, scalar h=1
ENG = ["v", "s"] * 16

_CACHE = {}


def _build():
    import concourse.bacc as bacc
    import concourse.mybir as mybir
    import concourse.tile as tile

    nc = bacc.Bacc("TRN2", target_bir_lowering=False, debug=False,
                   num_devices=NCORES, enable_partition_id=False)

    scl = nc.declare_dram_parameter("scl", [P, 4], mybir.dt.float32,
                                    isOutput=False)
    ez = nc.declare_dram_parameter("ez", [P, CAPZ], mybir.dt.bfloat16,
                                   isOutput=False)
    wz = nc.declare_dram_parameter("wz", [P, D_PROJ], mybir.dt.bfloat16,
                                   isOutput=False)
    e1e = nc.declare_dram_parameter("e1e", [P, 2, CAP1], mybir.dt.bfloat16,
                                    isOutput=False)
    w1 = nc.declare_dram_parameter("w1", [P, 2, D_PROJ], mybir.dt.bfloat16,
                                   isOutput=False)
    e0e = nc.declare_dram_parameter("e0e", [P, 8, CAP0], mybir.dt.bfloat16,
                                    isOutput=False)
    w0 = nc.declare_dram_parameter("w0", [P, 8, D_PROJ], mybir.dt.bfloat16,
                                   isOutput=False)
    out_t = nc.declare_dram_parameter("out_t", [OUT_ROWS, D_PROJ],
                                      mybir.dt.int8, isOutput=True)

    COPY = mybir.ActivationFunctionType.Copy

    with tile.TileContext(nc) as tc:
        with (
            tc.tile_pool(name="inp", bufs=1) as ipool,
            tc.tile_pool(name="psum", bufs=8, space="PSUM") as ppool,
            tc.tile_pool(name="ostage", bufs=4) as opool,
        ):
            ezt = ipool.tile([P, CAPZ], mybir.dt.bfloat16, tag="ez")
            wzt = ipool.tile([P, D_PROJ], mybir.dt.bfloat16, tag="wz")
            sct = ipool.tile([P, 4], mybir.dt.float32, tag="scl")
            e1t = ipool.tile([P, 2, CAP1], mybir.dt.bfloat16, tag="e1")
            w1t = ipool.tile([P, 2, D_PROJ], mybir.dt.bfloat16, tag="w1")
            e0t = ipool.tile([P, 8, CAP0], mybir.dt.bfloat16, tag="e0")
            w0t = ipool.tile([P, 8, D_PROJ], mybir.dt.bfloat16, tag="w0")
            wmt = ipool.tile([P, 640], mybir.dt.bfloat16, tag="wm")
            junk = ipool.tile([P, 16], mybir.dt.int8, tag="junk")

            # --- engine-path prewarm
            nc.vector.memset(wmt[:], 0)
            nc.vector.tensor_scalar_mul(junk[:, 0:8], wmt[:, 0:8], 2.0)
            nc.gpsimd.tensor_scalar_mul(junk[:, 8:16], wmt[:, 8:16], 2.0)

            # PE warmup bridges preamble end -> first data so the HAM
            # activity window is continuously busy (else first real
            # matmuls run at 1.2GHz)
            wps = ppool.tile([P, 512], mybir.dt.float32, tag="ps")
            for _ in range(4):
                nc.tensor.matmul(wps[:], wmt[:, 0:P], wmt[:, P:640],
                                 start=True, stop=True)

            # --- z-critical input chunks first on the sync ring
            nc.sync.dma_start(out=ezt[:, 0:P], in_=ez[:, 0:P])
            nc.sync.dma_start(out=wzt[:, 0:512], in_=wz[:, 0:512])
            nc.sync.dma_start(out=wzt[:, 512:1024], in_=wz[:, 512:1024])
            nc.sync.dma_start(out=sct[:], in_=scl[:])
            nc.sync.dma_start(out=ezt[:, P:512], in_=ez[:, P:512])
            nc.sync.dma_start(out=ezt[:, 512:1152], in_=ez[:, 512:1152])
            nc.sync.dma_start(out=ezt[:, 1152:CAPZ], in_=ez[:, 1152:CAPZ])
            # b1 inputs + ACT-table prewarm on the scalar queue/ring
            nc.scalar.dma_start(out=e1t[:], in_=e1e[:])
            nc.scalar.activation(junk[:, 0:8], wmt[:, 0:8], COPY, scale=2.0)
            nc.scalar.dma_start(out=w1t[:], in_=w1[:])
            # w0 halves + e0 are issued mid-loop (below) so their transfers
            # trail the z-critical ones

            out_v = out_t.rearrange("(p t) n -> p t n", t=16)

            ei = 0
            for gi in range(NGROUPS):
                ot = opool.tile([P, 2, D_PROJ], mybir.dt.int8, tag="o")
                for s in range(2):
                    t = ORDER[gi * 2 + s]
                    for h in range(2):
                        c0 = h * 512
                        ps = ppool.tile([P, 512], mybir.dt.float32, tag="ps")
                        if t < 14:
                            nc.tensor.matmul(
                                ps[:], ezt[:, t * P:(t + 1) * P],
                                wzt[:, c0:c0 + 512], start=True, stop=True)
                            sc = 0
                        elif t == 14:
                            for k in range(2):
                                nc.tensor.matmul(
                                    ps[:], e1t[:, k, :],
                                    w1t[:, k, c0:c0 + 512],
                                    start=(k == 0), stop=(k == 1))
                            sc = 1
                        else:
                            for q in range(8):
                                nc.tensor.matmul(
                                    ps[:], e0t[:, q, :],
                                    w0t[:, q, c0:c0 + 512],
                                    start=(q == 0), stop=(q == 7))
                            sc = 2
                        eng = ENG[ei]
                        ei += 1
                        if eng == "v":
                            nc.vector.tensor_scalar_mul(
                                ot[:, s, c0:c0 + 512], ps[:],
                                sct[:, sc:sc + 1])
                        else:
                            nc.scalar.activation(
                                ot[:, s, c0:c0 + 512], ps[:], COPY,
                                scale=sct[:, sc:sc + 1])
                nc.sync.dma_start(out=out_v[:, gi * 2:gi * 2 + 2, :],
                                  in_=ot[:])
                # deferred big-weight transfers, behind the z-critical ones
                if gi == 0:
                    nc.scalar.dma_start(out=w0t[:, 0:4, :], in_=w0[:, 0:4, :])
                elif gi == 1:
                    nc.scalar.dma_start(out=w0t[:, 4:8, :], in_=w0[:, 4:8, :])
                elif gi == 2:
                    nc.sync.dma_start(out=e0t[:], in_=e0e[:])
    nc.compile()
    return nc


def _route(flat):
    """Per-core token lists per segment (0=b0, 1=b1, 2=z)."""
    b_of = np.searchsorted(np.asarray(CUT[1:-1]), flat, side="right")
    per_core = [dict() for _ in range(NCORES)]
    for b in range(4):
        tb = np.nonzero(b_of == b)[0]
        lb = (flat[tb] - CUT[b]).astype(np.int64)
        seg = b if b < 2 else 2
        for c in range(NCORES):
            per_core[c].setdefault(seg, []).append(
                (b, tb[c::NCORES], lb[c::NCORES]))
    return per_core


def _ensure_trace_shim():
    import sys, types
    try:
        import antenv.axon_hooks  # noqa: F401
    except Exception:
        try:
            import antenv
            mod = types.ModuleType("antenv.axon_hooks")
            mod.get_axon_ntff_profile_hook = lambda: None
            mod.set_axon_ntff_profile_hook = lambda h: None
            sys.modules["antenv.axon_hooks"] = mod
            antenv.axon_hooks = mod
        except Exception:
            pass


def kernel(inp, emb0, emb1, emb2, emb3, proj0, proj1, proj2, proj3):
    _ensure_trace_shim()
    from concourse.bass_utils import run_bass_kernel_spmd

    embs = [np.asarray(emb0), np.asarray(emb1), np.asarray(emb2),
            np.asarray(emb3)]
    projs_in = [np.asarray(proj0), np.asarray(proj1), np.asarray(proj2),
                np.asarray(proj3)]
    inp = np.asarray(inp)
    flat = inp.reshape(-1).astype(np.int64)
    N = flat.shape[0]

    per_core = _route(flat)
    fallback = []

    w0 = np.ascontiguousarray(
        projs_in[0].T.reshape(8, P, D_PROJ).transpose(1, 0, 2)).astype(BF16)
    w1 = np.ascontiguousarray(
        projs_in[1].T.reshape(2, P, D_PROJ).transpose(1, 0, 2)).astype(BF16)
    wzf = np.zeros((P, D_PROJ), np.float32)
    wzf[0:64] = projs_in[2].T
    wzf[64:80] = projs_in[3].T
    wz = wzf.astype(BF16)

    # per-region int8 scales from output-sigma estimates (z uses b2's sigma)
    sig = [float(embs[b].std()) * float(projs_in[b].std())
           * np.sqrt(D_EMBS[b]) for b in range(4)]
    S = np.array([127.0 / (NSIG * sig[2]),
                  127.0 / (NSIG * sig[1]),
                  127.0 / (NSIG * sig[0]), 1.0], np.float32)
    scl = np.broadcast_to(S, (P, 4)).copy()
    slot_arr = np.array([SLOT[t] for t in range(16)], np.int64)
    inv_seg = {2: 1.0 / S[0], 1: 1.0 / S[1], 0: 1.0 / S[2]}

    caps = {0: CAP0, 1: CAP1, 2: CAPZ}
    base_tile = {2: 0, 1: 14, 0: 15}
    in_maps = []
    core_rows = []
    for c in range(NCORES):
        e1h = np.zeros((P, 2, CAP1), BF16)
        e0h = np.zeros((P, 8, CAP0), BF16)
        ez = np.zeros((P, CAPZ), BF16)
        rows, toks, scas = [], [], []
        for seg, parts in per_core[c].items():
            cap = caps[seg]
            col = 0
            for (b, tb, lb) in parts:
                n = len(tb)
                keep = min(n, cap - col)
                if keep < n:
                    for t, r in zip(tb[keep:], lb[keep:]):
                        fallback.append((int(t), b, int(r)))
                    tb, lb = tb[:keep], lb[:keep]
                if keep == 0:
                    continue
                g = embs[b][lb].astype(BF16)          # [keep, d_b]
                if seg == 0:
                    e0h[:, :, col:col + keep] = \
                        g.T.reshape(8, P, keep).transpose(1, 0, 2)
                elif seg == 1:
                    e1h[:, :, col:col + keep] = \
                        g.T.reshape(2, P, keep).transpose(1, 0, 2)
                else:
                    if b == 2:
                        ez[0:64, col:col + keep] = g.T
                    else:
                        ez[64:80, col:col + keep] = g.T
                gcol = col + np.arange(keep)
                rows.append((gcol % P) * 16
                            + slot_arr[base_tile[seg] + gcol // P])
                toks.append(tb)
                scas.append(np.full(keep, inv_seg[seg], np.float32))
                col += keep
        core_rows.append((np.concatenate(rows), np.concatenate(toks),
                          np.concatenate(scas)))
        in_maps.append({"scl": scl, "ez": ez, "e1e": e1h, "e0e": e0h,
                        "w0": w0, "w1": w1, "wz": wz})

    if "nc" not in _CACHE:
        _CACHE["nc"] = _build()
    nc = _CACHE["nc"]

    res = run_bass_kernel_spmd(nc, in_maps, core_ids=list(range(NCORES)))
    _CACHE["last_result"] = res

    final = np.zeros((N, D_PROJ), np.float32)
    for c in range(NCORES):
        slab = res.results[c]["out_t"].astype(np.float32)  # [OUT_ROWS, 1024]
        rows, toks, scas = core_rows[c]
        final[toks] = slab[rows] * scas[:, None]

    for (t, b, r) in fallback:
        final[t] = embs[b][r].astype(np.float32) @ projs_in[b].T

    return final.reshape(*inp.shape, D_PROJ)


# revision 13
# speedup vs baseline: 1.1647x; 1.1647x over previous
"""AdaptiveEmbedding kernel for 8 TRN2 NeuronCores (host-gather GEMM,
int8 output). v6: all-bf16, staged input chunks, deferred weight DMAs,
half-tile casts on vector+scalar, 8 PSUM banks, 8 output groups.

Host routes tokens to vocab buckets and gathers their embedding rows into
dense feature-on-partition tiles (token-parallel across 8 cores, projection
weights replicated). Device is a pure pipelined GEMM: per 128-token tile,
stationary = gathered embeddings [K_feat, 128tok], moving = projection
[K_feat, 512 dproj], PSUM [tok, 512] f32 per half, then a scaled
round+saturate cast to int8, DMA out in p-major layout (row = p*16+slot,
2KB contiguous per partition per 2-tile group). Host dequantizes and
scatters rows back to token order.

Norm note: bucket 0 tokens carry ~54% of the output norm (d_emb=1024),
so fp8 there (3.7% rel) costs ~2.4e-2 global -- everything stays bf16.

Scheduling: the z-critical transfers (ez first chunk, wz halves) issue
first on the sync HWDGE ring; e1/w1 go on the scalar ring; the big w0
halves and e0 are issued mid-loop (after groups 0/1/2) so their transfers
never contend with the z stream. PE warmup matmuls bridge the queue from
preamble end (~7.7us) to first data (~9.4us) to keep the HAM activity
window warm. gpsimd issues no DMAs (SWDGE quiesces against all in-flight
DMA) and cannot read PSUM, so casts alternate vector/scalar per 512-col
half-tile with 8 single-bank PSUM tiles in flight.

Routing overflow beyond the static caps falls back to exact numpy on host.
Self-contained: shapes hardcoded.
"""

import numpy as np
import ml_dtypes

BF16 = ml_dtypes.bfloat16

CUT = [0, 20000, 40000, 200000, 267735]
D_EMBS = [1024, 256, 64, 16]
D_PROJ = 1024
NCORES = 8
P = 128

CAP0 = 128     # b0: mean 153 — overflow (~25/core) goes to exact host fallback
CAP1 = 128     # b1: same
CAPZ = 1792    # b2+b3 merged: mean 1742, sigma ~16; 14 full 128-token tiles
NSIG = 5.0     # quantization range in output sigmas

# tile ids: 0..13 = z tiles, 14 = b1, 15 = b0. b1 sits mid-stream; b0 runs
# LAST so its (late-arriving, largest) weight stream never stalls the
# in-order tensor queue.
ORDER = [0, 1, 2, 3,  4, 5, 14, 6,  7, 8, 9, 10,  11, 12, 13, 15]
# output groups as (start slot, n tiles): small final groups so the tail
# (last cast -> out DMA -> sem) is short
GROUPS = [(0, 2), (2, 2), (4, 2), (6, 2), (8, 2), (10, 2), (12, 2),
          (14, 1), (15, 1)]
SLOT = {t: s for s, t in enumerate(ORDER)}   # tile id -> slot
OUT_ROWS = 16 * P                            # 2048

# cast engine per 512-col half-tile: vector takes h=0, scalar h=1
ENG = ["v", "s"] * 16

_CACHE = {}


def _build():
    import concourse.bacc as bacc
    import concourse.mybir as mybir
    import concourse.tile as tile

    nc = bacc.Bacc("TRN2", target_bir_lowering=False, debug=False,
                   num_devices=NCORES, enable_partition_id=False)

    scl = nc.declare_dram_parameter("scl", [P, 4], mybir.dt.float32,
                                    isOutput=False)
    ez = nc.declare_dram_parameter("ez", [P, CAPZ], mybir.dt.bfloat16,
                                   isOutput=False)
    wz = nc.declare_dram_parameter("wz", [P, D_PROJ], mybir.dt.bfloat16,
                                   isOutput=False)
    e1e = nc.declare_dram_parameter("e1e", [P, 2, CAP1], mybir.dt.bfloat16,
                                    isOutput=False)
    w1 = nc.declare_dram_parameter("w1", [P, 2, D_PROJ], mybir.dt.bfloat16,
                                   isOutput=False)
    e0e = nc.declare_dram_parameter("e0e", [P, 8, CAP0], mybir.dt.bfloat16,
                                    isOutput=False)
    w0 = nc.declare_dram_parameter("w0", [P, 8, D_PROJ], mybir.dt.bfloat16,
                                   isOutput=False)
    out_t = nc.declare_dram_parameter("out_t", [OUT_ROWS, D_PROJ],
                                      mybir.dt.int8, isOutput=True)

    COPY = mybir.ActivationFunctionType.Copy

    with tile.TileContext(nc) as tc:
        with (
            tc.tile_pool(name="inp", bufs=1) as ipool,
            tc.tile_pool(name="psum", bufs=8, space="PSUM") as ppool,
            tc.tile_pool(name="ostage", bufs=4) as opool,
        ):
            ezt = ipool.tile([P, CAPZ], mybir.dt.bfloat16, tag="ez")
            wzt = ipool.tile([P, D_PROJ], mybir.dt.bfloat16, tag="wz")
            sct = ipool.tile([P, 4], mybir.dt.float32, tag="scl")
            e1t = ipool.tile([P, 2, CAP1], mybir.dt.bfloat16, tag="e1")
            w1t = ipool.tile([P, 2, D_PROJ], mybir.dt.bfloat16, tag="w1")
            e0t = ipool.tile([P, 8, CAP0], mybir.dt.bfloat16, tag="e0")
            w0t = ipool.tile([P, 8, D_PROJ], mybir.dt.bfloat16, tag="w0")
            wmt = ipool.tile([P, 640], mybir.dt.bfloat16, tag="wm")
            junk = ipool.tile([P, 16], mybir.dt.int8, tag="junk")

            # --- engine-path prewarm
            nc.vector.memset(wmt[:], 0)
            nc.vector.tensor_scalar_mul(junk[:, 0:8], wmt[:, 0:8], 2.0)
            nc.gpsimd.tensor_scalar_mul(junk[:, 8:16], wmt[:, 8:16], 2.0)

            # PE warmup bridges preamble end -> first data AND must be a
            # continuous >=3.4us busy stretch so the HAM activity window
            # flips the PE clock gate to 2.4GHz (8 cold matmuls x ~430ns);
            # too-short warmup leaves the whole stream at 1.2GHz
            wps = ppool.tile([P, 512], mybir.dt.float32, tag="ps")
            for _ in range(8):
                nc.tensor.matmul(wps[:], wmt[:, 0:P], wmt[:, P:640],
                                 start=True, stop=True)

            # --- input DMAs. sync ring carries only the z-critical chunks
            # (plus the 8 output groups later, so no head-of-line waits);
            # scalar ring carries every weight/bucket stream.
            nc.sync.dma_start(out=ezt[:, 0:P], in_=ez[:, 0:P])
            nc.sync.dma_start(out=wzt[:, 0:512], in_=wz[:, 0:512])
            nc.sync.dma_start(out=wzt[:, 512:1024], in_=wz[:, 512:1024])
            nc.sync.dma_start(out=sct[:], in_=scl[:])
            nc.sync.dma_start(out=ezt[:, P:512], in_=ez[:, P:512])
            nc.sync.dma_start(out=ezt[:, 512:1152], in_=ez[:, 512:1152])
            nc.sync.dma_start(out=ezt[:, 1152:CAPZ], in_=ez[:, 1152:CAPZ])
            nc.scalar.dma_start(out=e1t[:], in_=e1e[:])
            nc.scalar.dma_start(out=w1t[:], in_=w1[:])
            nc.scalar.dma_start(out=w0t[:, 0:4, :], in_=w0[:, 0:4, :])
            nc.scalar.dma_start(out=w0t[:, 4:8, :], in_=w0[:, 4:8, :])
            nc.scalar.dma_start(out=e0t[:], in_=e0e[:])
            # ACT-table prewarm before the first scalar cast
            nc.scalar.activation(junk[:, 0:8], wmt[:, 0:8], COPY, scale=2.0)

            out_v = out_t.rearrange("(p t) n -> p t n", t=16)

            ei = 0
            for gi, (s0, gn) in enumerate(GROUPS):
                ot = opool.tile([P, gn, D_PROJ], mybir.dt.int8, tag=f"o{gn}")
                for s in range(gn):
                    t = ORDER[s0 + s]
                    for h in range(2):
                        c0 = h * 512
                        ps = ppool.tile([P, 512], mybir.dt.float32, tag="ps")
                        if t < 14:
                            nc.tensor.matmul(
                                ps[:], ezt[:, t * P:(t + 1) * P],
                                wzt[:, c0:c0 + 512], start=True, stop=True)
                            sc = 0
                        elif t == 14:
                            for k in range(2):
                                nc.tensor.matmul(
                                    ps[:], e1t[:, k, :],
                                    w1t[:, k, c0:c0 + 512],
                                    start=(k == 0), stop=(k == 1))
                            sc = 1
                        else:
                            for q in range(8):
                                nc.tensor.matmul(
                                    ps[:], e0t[:, q, :],
                                    w0t[:, q, c0:c0 + 512],
                                    start=(q == 0), stop=(q == 7))
                            sc = 2
                        eng = ENG[ei]
                        ei += 1
                        if eng == "v":
                            nc.vector.tensor_scalar_mul(
                                ot[:, s, c0:c0 + 512], ps[:],
                                sct[:, sc:sc + 1])
                        else:
                            nc.scalar.activation(
                                ot[:, s, c0:c0 + 512], ps[:], COPY,
                                scale=sct[:, sc:sc + 1])
                nc.sync.dma_start(out=out_v[:, s0:s0 + gn, :], in_=ot[:])
    nc.compile()
    return nc


def _route(flat):
    """Per-core token lists per segment (0=b0, 1=b1, 2=z)."""
    b_of = np.searchsorted(np.asarray(CUT[1:-1]), flat, side="right")
    per_core = [dict() for _ in range(NCORES)]
    for b in range(4):
        tb = np.nonzero(b_of == b)[0]
        lb = (flat[tb] - CUT[b]).astype(np.int64)
        seg = b if b < 2 else 2
        for c in range(NCORES):
            per_core[c].setdefault(seg, []).append(
                (b, tb[c::NCORES], lb[c::NCORES]))
    return per_core


def _ensure_trace_shim():
    import sys, types
    try:
        import antenv.axon_hooks  # noqa: F401
    except Exception:
        try:
            import antenv
            mod = types.ModuleType("antenv.axon_hooks")
            mod.get_axon_ntff_profile_hook = lambda: None
            mod.set_axon_ntff_profile_hook = lambda h: None
            sys.modules["antenv.axon_hooks"] = mod
            antenv.axon_hooks = mod
        except Exception:
            pass


def kernel(inp, emb0, emb1, emb2, emb3, proj0, proj1, proj2, proj3):
    _ensure_trace_shim()
    from concourse.bass_utils import run_bass_kernel_spmd

    embs = [np.asarray(emb0), np.asarray(emb1), np.asarray(emb2),
            np.asarray(emb3)]
    projs_in = [np.asarray(proj0), np.asarray(proj1), np.asarray(proj2),
                np.asarray(proj3)]
    inp = np.asarray(inp)
    flat = inp.reshape(-1).astype(np.int64)
    N = flat.shape[0]

    per_core = _route(flat)
    fallback = []

    w0 = np.ascontiguousarray(
        projs_in[0].T.reshape(8, P, D_PROJ).transpose(1, 0, 2)).astype(BF16)
    w1 = np.ascontiguousarray(
        projs_in[1].T.reshape(2, P, D_PROJ).transpose(1, 0, 2)).astype(BF16)
    wzf = np.zeros((P, D_PROJ), np.float32)
    wzf[0:64] = projs_in[2].T
    wzf[64:80] = projs_in[3].T
    wz = wzf.astype(BF16)

    # per-region int8 scales from output-sigma estimates (z uses b2's sigma)
    sig = [float(embs[b].std()) * float(projs_in[b].std())
           * np.sqrt(D_EMBS[b]) for b in range(4)]
    S = np.array([127.0 / (NSIG * sig[2]),
                  127.0 / (NSIG * sig[1]),
                  127.0 / (NSIG * sig[0]), 1.0], np.float32)
    scl = np.broadcast_to(S, (P, 4)).copy()
    slot_arr = np.array([SLOT[t] for t in range(16)], np.int64)
    inv_seg = {2: 1.0 / S[0], 1: 1.0 / S[1], 0: 1.0 / S[2]}

    caps = {0: CAP0, 1: CAP1, 2: CAPZ}
    base_tile = {2: 0, 1: 14, 0: 15}
    in_maps = []
    core_rows = []
    for c in range(NCORES):
        e1h = np.zeros((P, 2, CAP1), BF16)
        e0h = np.zeros((P, 8, CAP0), BF16)
        ez = np.zeros((P, CAPZ), BF16)
        rows, toks, scas = [], [], []
        for seg, parts in per_core[c].items():
            cap = caps[seg]
            col = 0
            for (b, tb, lb) in parts:
                n = len(tb)
                keep = min(n, cap - col)
                if keep < n:
                    for t, r in zip(tb[keep:], lb[keep:]):
                        fallback.append((int(t), b, int(r)))
                    tb, lb = tb[:keep], lb[:keep]
                if keep == 0:
                    continue
                g = embs[b][lb].astype(BF16)          # [keep, d_b]
                if seg == 0:
                    e0h[:, :, col:col + keep] = \
                        g.T.reshape(8, P, keep).transpose(1, 0, 2)
                elif seg == 1:
                    e1h[:, :, col:col + keep] = \
                        g.T.reshape(2, P, keep).transpose(1, 0, 2)
                else:
                    if b == 2:
                        ez[0:64, col:col + keep] = g.T
                    else:
                        ez[64:80, col:col + keep] = g.T
                gcol = col + np.arange(keep)
                rows.append((gcol % P) * 16
                            + slot_arr[base_tile[seg] + gcol // P])
                toks.append(tb)
                scas.append(np.full(keep, inv_seg[seg], np.float32))
                col += keep
        core_rows.append((np.concatenate(rows), np.concatenate(toks),
                          np.concatenate(scas)))
        in_maps.append({"scl": scl, "ez": ez, "e1e": e1h, "e0e": e0h,
                        "w0": w0, "w1": w1, "wz": wz})

    if "nc" not in _CACHE:
        _CACHE["nc"] = _build()
    nc = _CACHE["nc"]

    res = run_bass_kernel_spmd(nc, in_maps, core_ids=list(range(NCORES)))
    _CACHE["last_result"] = res

    final = np.zeros((N, D_PROJ), np.float32)
    for c in range(NCORES):
        slab = res.results[c]["out_t"].astype(np.float32)  # [OUT_ROWS, 1024]
        rows, toks, scas = core_rows[c]
        final[toks] = slab[rows] * scas[:, None]

    for (t, b, r) in fallback:
        final[t] = embs[b][r].astype(np.float32) @ projs_in[b].T

    return final.reshape(*inp.shape, D_PROJ)
